# revision 3
# baseline (speedup 1.0000x reference)
"""MixLinear int4-GEMM kernel for 8x TRN2 NeuronCores.

Sharding: 4-way over rows (M) x 2-way over out_features; each core gets
rows shard [2048, 4096] and a 2048-wide out-feature shard. The main loop
is software-pipelined (load | quantize | matmul+store stages with lag) so
the per-tile chain overlaps across engines; x loads are spread over the
three DMA rings (Act/Pool/SP hwdge+swdge dispatch queues).

Per core, per 128-row tile:
  1. GPSIMD ap_gather pulls the 256 outlier activations; DVE zeroes the
     outlier columns in place (x * mask, bf16 mask) and an abs-max reduce
     gives the exact masked row scale s = max|x_kept|/7, r = 1/s.
  2. ScalarE quantizes with the magic-number RNE trick (two activation
     passes: t = x*r + 1.5*2^23 in f32, q = t - magic in bf16), with
     rowsum(q) accumulated for free via accum_out.
  3. q | aos (outlier activations * r) are packed in one [128, 4352] bf16
     tile, transposed by a single DMA-xbar, and the 32 int k-tiles
     converted to fp8e4.
  4. Int GEMM runs in fp8 DoubleRow mode (2 k-tiles per instruction, ~2x
     bf16 PE rate). Weights are stored as w' = nibble^8 = w+8 in [0,15]
     (exact in fp8); the -8 offset is folded into the dequant bias
     -8*rowsum(q)*s. Outlier GEMM stays bf16 against weight_cache/sc.
  5. Dequant: ScalarE psum evict (psum*s + bias -> bf16), DVE multiply by
     scale_col (bf16), DMA out; the host upcasts bf16 -> f32.
"""

import os as _os

import numpy as np

B, S, IN, OUT, FP = 4, 2048, 4096, 4096, 256
M = B * S
NCORES = 8
MSPLIT, OSPLIT = (
    (2, 4) if _os.environ.get("KERNEL_SHARD", "4x2") == "2x4" else (4, 2)
)
MSH = M // MSPLIT    # rows per core
OS = OUT // OSPLIT   # out-features per core
QMAX = 7.0
MAGIC = 12582912.0   # 1.5 * 2**23
P = 128
KT = IN // P         # 32 int k-tiles
NKT = KT + 2         # + 2 outlier (aos) k-tiles
KEPT = IN - FP       # 3840


def emit_core_kernel(nc, tc, ne_even):
    import concourse.bass as bass  # noqa: F401
    import concourse.mybir as mybir

    f32 = mybir.dt.float32
    bf16 = mybir.dt.bfloat16
    f8 = mybir.dt.float8e4
    i32 = mybir.dt.int32
    i16 = mybir.dt.int16
    Alu = mybir.AluOpType
    Act = mybir.ActivationFunctionType
    DR = mybir.MatmulPerfMode.DoubleRow

    MT = MSH // P            # 16 row tiles
    OJ = OS // P             # 16 out-feature chunks of 128
    MMW = int(_os.environ.get("KERNEL_MMW", "512"))  # matmul rhs width
    NB = OS // MMW           # matmul column blocks
    BF16 = _os.environ.get("KERNEL_BF16", "0") == "1"  # disable fp8 path
    CONV = _os.environ.get("KERNEL_CONV", "dve")  # qT fp8 convert engine
    wdt = bf16 if BF16 else f8

    x = nc.dram_tensor("x", [MSH, IN], f32, kind="ExternalInput")
    qw = nc.dram_tensor("qw", [OS, IN // 2], mybir.dt.uint8, kind="ExternalInput")
    sc = nc.dram_tensor("sc", [OS], f32, kind="ExternalInput")
    wc = nc.dram_tensor("wc", [OS, FP], f32, kind="ExternalInput")
    idx_ao = nc.dram_tensor("idx_ao", [P, FP // 16], i16, kind="ExternalInput")
    maskrow = nc.dram_tensor("maskrow", [IN], bf16, kind="ExternalInput")
    y = nc.dram_tensor("y", [MSH, OS], bf16, kind="ExternalOutput")

    from concourse import library_config

    nc.gpsimd.load_library(library_config.ap_gather)

    with tc.tile_pool(name="persist", bufs=1) as pers:
        # persistent tiles
        wT8 = pers.tile([P, KT, OS], wdt)            # int weights, k-major
        wcT = pers.tile([P, 2 * OJ, P], bf16)        # outlier fp weights (interleaved)
        sc_bf = pers.tile([P, OS], bf16)             # scale_col broadcast
        ia = pers.tile([P, FP // 16], i16)
        nc.sync.dma_start(ia[:], idx_ao[:])
        maskF = pers.tile([P, IN], bf16)
        nc.sync.dma_start(maskF[:], maskrow[None, :].to_broadcast((P, IN)))

        # ------------------------- setup phase -------------------------
        with (
            tc.tile_pool(name="wstage", bufs=2) as ws,
            tc.tile_pool(name="wstage1", bufs=1) as ws1,
        ):
            # scale_col broadcast -> bf16
            scb32 = ws1.tile([P, OS], f32)
            nc.sync.dma_start(scb32[:], sc[None, :].to_broadcast((P, OS)))
            nc.vector.tensor_copy(sc_bf[:], scb32[:])

            # per-partition scale_col view + reciprocal (for wc prescale)
            scp = ws1.tile([P, OJ], f32)
            nc.sync.dma_start(scp[:], sc.rearrange("(j p) -> p j", p=P))
            rscp = ws1.tile([P, OJ], f32)
            nc.vector.reciprocal(rscp[:], scp[:])

            # weight_cache: wcp[o, f] = wc[o, f] / sc[o]  (ScalarE, bf16 out)
            wc_sb = ws1.tile([P, OJ, FP], f32)
            nc.sync.dma_start(wc_sb[:], wc.rearrange("(j p) f -> p j f", p=P))
            wcp = ws1.tile([P, OJ, FP], bf16)
            for j in range(OJ):
                nc.scalar.activation(
                    wcp[:, j, :], wc_sb[:, j, :], Act.Copy, scale=rscp[:, j : j + 1]
                )
            # one xbar: [128, OJ*FP] -> [128, OJ*FP/128, 128]; tile t=2j+fh
            nc.sync.dma_start_transpose(wcT[:], wcp[:])

            # int4 weights + gathered outlier rows, per 128-out-channel chunk
            qw_v = qw.rearrange("(j p) k -> p j k", p=P)
            for j in range(OJ):
                qwj = ws.tile([P, IN // 2], i32, tag="qwj")
                # u8 -> i32 widening cast happens inside the (gpsimd) DMA
                nc.gpsimd.dma_start(qwj[:], qw_v[:, j, :])
                wtmp = ws.tile([P, IN // 2, 2], i32, tag="wtmp")
                # low nibble -> even cols: (v & 15) ^ 8  (= w + 8 in [0,15])
                nc.vector.tensor_scalar(
                    wtmp[:, :, 0], qwj[:], 15, 8, Alu.bitwise_and, Alu.bitwise_xor
                )
                # high nibble -> odd cols: (v >> 4) ^ 8
                nc.vector.tensor_scalar(
                    wtmp[:, :, 1], qwj[:], 4, 8, Alu.arith_shift_right, Alu.bitwise_xor
                )
                w_ok = ws.tile([P, IN // 2, 2], bf16, tag="wok")
                nc.vector.tensor_copy(w_ok[:], wtmp[:])
                # transpose to k-major and convert to fp8
                wtb = ws.tile([P, KT, P], bf16, tag="wtb")
                nc.sync.dma_start_transpose(wtb[:], w_ok[:])
                nc.scalar.activation(
                    wT8[:, :, j * P : (j + 1) * P], wtb[:], Act.Copy
                )


        # ------------------------- main loop -------------------------
        with (
            tc.tile_pool(name="xp", bufs=4) as xp,
            tc.tile_pool(name="qp", bufs=1) as qp,
            tc.tile_pool(name="qtp", bufs=2) as qtp,
            tc.tile_pool(name="qt8p", bufs=2) as qt8p,
            tc.tile_pool(name="aop", bufs=2) as aop,
            tc.tile_pool(name="sp", bufs=4) as sp,
            tc.tile_pool(name="yp", bufs=2) as yp,
            tc.tile_pool(name="pp", bufs=2, space="PSUM") as pp,
        ):
            state = {}

            def emit_load(mi):
                x_t = xp.tile([P, IN], f32)
                rows = slice(mi * P, (mi + 1) * P)
                # x spread over all three DMA rings
                nc.scalar.dma_start(x_t[:, :1408], x[rows, :1408])
                nc.gpsimd.dma_start(x_t[:, 1408:2816], x[rows, 1408:2816])
                nc.sync.dma_start(x_t[:, 2816:], x[rows, 2816:])
                state[("x", mi)] = x_t

            def emit_quant(mi):
                x_t = state.pop(("x", mi))
                # outlier activations first (gather cost ~25ns/idx)
                aot = aop.tile([P, FP], f32, tag="ao")
                nc.gpsimd.ap_gather(
                    aot[:, :, None], x_t[:, :, None], ia[:],
                    channels=P, num_elems=IN, d=1, num_idxs=FP,
                )
                ao = aot[:]
                # zero outlier columns in place, then plain abs-max reduce
                nc.vector.tensor_tensor(x_t[:], x_t[:], maskF[:], Alu.mult)
                mx = sp.tile([P, 1], f32, tag="mx")
                nc.vector.tensor_reduce(
                    mx[:], x_t[:], mybir.AxisListType.X, Alu.max,
                    apply_absolute_value=True,
                )
                s_t = sp.tile([P, 1], f32, tag="s")
                nc.vector.tensor_scalar(
                    s_t[:], mx[:], float(np.float32(1.0) / np.float32(QMAX)),
                    None, Alu.mult,
                )
                r_t = sp.tile([P, 1], f32, tag="r")
                nc.vector.reciprocal(r_t[:], s_t[:])

                # quantize: t = x*r + MAGIC (f32), q = t - MAGIC (bf16 + rowsum)
                nc.scalar.activation(
                    x_t[:], x_t[:], Act.Copy, bias=MAGIC, scale=r_t[:]
                )
                qa = qp.tile([P, NKT * P], bf16)
                rsq = sp.tile([P, 1], f32, tag="rsq")
                nc.scalar.activation(
                    qa[:, :IN], x_t[:], Act.Copy, bias=-MAGIC, accum_out=rsq[:]
                )
                # outlier activations: aos = ao * r (bf16)
                nc.scalar.activation(
                    qa[:, IN :], ao, Act.Copy, scale=r_t[:]
                )

                # dequant bias: -8 * rsq * s
                s8 = sp.tile([P, 1], f32, tag="s8")
                nc.vector.tensor_scalar(s8[:], s_t[:], -8.0, None, Alu.mult)
                bias_t = sp.tile([P, 1], f32, tag="bias")
                nc.vector.tensor_tensor(bias_t[:], rsq[:], s8[:], Alu.mult)

                # transpose all operands in one xbar; convert first 34 to fp8
                qT = qtp.tile([P, NKT, P], bf16)
                nc.sync.dma_start_transpose(qT[:], qa[:])
                if BF16:
                    qT8 = qT
                else:
                    qT8 = qt8p.tile([P, KT, P], f8)
                    if CONV == "act":
                        nc.scalar.activation(qT8[:], qT[:, :KT, :], Act.Copy)
                    elif CONV == "split":
                        nc.vector.tensor_copy(
                            qT8[:, : KT // 2, :], qT[:, : KT // 2, :]
                        )
                        nc.scalar.activation(
                            qT8[:, KT // 2 :, :], qT[:, KT // 2 : KT, :], Act.Copy,
                        )
                    else:
                        nc.vector.tensor_copy(qT8[:], qT[:, :KT, :])
                state[("q", mi)] = (qT, qT8, s_t, bias_t)

            def emit_mm(mi):
                qT, qT8, s_t, bias_t = state.pop(("q", mi))
                # GEMMs: 16 fp8-DR int pairs + 1 DR correction + 2 bf16 outlier
                psum = pp.tile([P, OS], f32)
                JB = MMW // P  # out-feature chunks per column block
                for b in range(NB):
                    cs = slice(b * MMW, (b + 1) * MMW)
                    if BF16:
                        for ko in range(KT):
                            nc.tensor.matmul(
                                psum[:, cs], qT8[:, ko, :], wT8[:, ko, cs],
                                start=(ko == 0), stop=False,
                            )
                    else:
                        for kp in range(KT // 2):
                            nc.tensor.matmul(
                                psum[:, cs],
                                qT8[:, 2 * kp : 2 * kp + 2, :],
                                wT8[:, 2 * kp : 2 * kp + 2, cs],
                                start=(kp == 0), stop=False, perf_mode=DR,
                            )
                    for fh in range(2):
                        nc.tensor.matmul(
                            psum[:, cs],
                            qT[:, KT + fh, :],
                            wcT[:, 2 * JB * b + fh : 2 * JB * b + fh + 2 * JB - 1 : 2, :],
                            start=False, stop=(fh == 1),
                        )

                # dequant + store (sc multiply in-place to save SBUF)
                t1 = yp.tile([P, OS], bf16, tag="t1")
                nc.scalar.activation(
                    t1[:], psum[:], Act.Identity, scale=s_t[:], bias=bias_t[:]
                )
                nc.vector.tensor_tensor(t1[:], t1[:], sc_bf[:], Alu.mult)
                nc.sync.dma_start(y[mi * P : (mi + 1) * P, :], t1[:])

            # software pipeline: load(t) | quant(t-2) | matmul+store(t-3)
            for t in range(MT + 3):
                if t < MT:
                    emit_load(t)
                if 0 <= t - 2 < MT:
                    emit_quant(t - 2)
                if 0 <= t - 3 < MT:
                    emit_mm(t - 3)

    return nc


def build_nc(ne_even):
    import concourse.bacc as bacc
    import concourse.tile as tile

    nc = bacc.Bacc(None, target_bir_lowering=False)
    with tile.TileContext(nc) as tc:
        emit_core_kernel(nc, tc, ne_even)
    nc.compile()
    return nc


def wrap_idx(v):
    w = np.asarray(v).astype(np.int16).reshape(-1, 16)
    return np.ascontiguousarray(np.tile(w.T, (8, 1)).astype(np.int16))


def make_host_inputs(x, q_weight, scale_col, weight_cache, ind):
    """Shard/relayout full inputs into per-core input maps (no arithmetic)."""
    xf = np.ascontiguousarray(x.reshape(M, IN).astype(np.float32, copy=False))
    ind = np.asarray(ind).astype(np.int64)
    evens = ind[ind % 2 == 0]
    odds = ind[ind % 2 == 1]
    ind_perm = np.concatenate([evens, odds])
    perm = np.concatenate([np.where(ind % 2 == 0)[0], np.where(ind % 2 == 1)[0]])
    kept = np.setdiff1d(np.arange(IN), ind)
    scf = np.asarray(scale_col).reshape(-1).astype(np.float32, copy=False)
    wcf = np.asarray(weight_cache).astype(np.float32, copy=False)[:, perm]

    import ml_dtypes
    idx_ao = wrap_idx(ind_perm)
    maskrow = np.ones(IN, dtype=ml_dtypes.bfloat16)
    maskrow[ind] = 0

    in_maps = []
    for c in range(NCORES):
        cm, co = c // OSPLIT, c % OSPLIT
        m0, o0 = cm * MSH, co * OS
        in_maps.append(
            {
                "x": xf[m0 : m0 + MSH],
                "qw": np.ascontiguousarray(q_weight[o0 : o0 + OS]).astype(
                    np.uint8
                ),
                "sc": np.ascontiguousarray(scf[o0 : o0 + OS]),
                "wc": np.ascontiguousarray(wcf[o0 : o0 + OS]),
                "idx_ao": idx_ao,
                "maskrow": maskrow,
            }
        )
    return in_maps, len(evens)


_NC_CACHE = {}


def kernel(x, q_weight, scale_col, weight_cache, ind, trace=False):
    from concourse.bass_utils import run_bass_kernel_spmd

    in_maps, ne_even = make_host_inputs(x, q_weight, scale_col, weight_cache, ind)
    key = ne_even
    if key not in _NC_CACHE:
        _NC_CACHE[key] = build_nc(ne_even)
    nc = _NC_CACHE[key]

    res = run_bass_kernel_spmd(nc, in_maps, list(range(NCORES)), trace=trace)
    yshards = [
        np.asarray(res.results[c]["y"]).astype(np.float32) for c in range(NCORES)
    ]
    rows = [
        np.concatenate(yshards[cm * OSPLIT : (cm + 1) * OSPLIT], axis=1)
        for cm in range(MSPLIT)
    ]
    yfull = np.concatenate(rows, axis=0).reshape(B, S, OUT)
    if trace:
        return yfull, res
    return yfull
